# revision 34
# baseline (speedup 1.0000x reference)
"""Distributed CLIP-style loss (l2i symmetric CE + g2i NT-Xent) on 8 TRN2 cores.

Strategy (v2): data-parallel row sharding with column-rotated per-core inputs
(rotation = the core's global row offset) so diagonals sit at static local
offsets and one SPMD program serves all 8 cores.

Key optimizations over v1:
 - z ships as fp8-e4m3 (4MB/core instead of 8MB bf16); the g2i GEMM runs in
   fp8 DoubleRow mode (2x PE throughput). Norms are computed on-device from
   fp8 squares via a DoubleRow ones-matmul.
 - g2i exploits symmetry of the similarity matrix: each 128-row tile computes
   only the column window [0, 2560) (its forward half, window-aligned across
   tiles); the backward half comes from column sums of the exp'd forward
   blocks (fp8 ones-matmul), assembled with the row sums on the host.
 - l2i computes the image@text GEMM once. Row-side LSE is computed on-device;
   the text-side (column) LSE partials come from f32 PE-transposes of the
   logits + per-column (max, sum) over the core's 256 rows, combined on host.
 - norm pipeline (fp8 squares -> ones-mm -> recip -> sqrt) is split across
   DVE/gpsimd and interleaved with phase A.
"""

import numpy as np
import ml_dtypes

import concourse.bass as bass
import concourse.mybir as mybir
from concourse.tile import TileContext
from concourse import bass_utils


# --- compat patches for the walrus build in this container ---------------
def _sem_clear_compat(self, sem):
    nums = list(sem) if isinstance(sem, range) else [
        sem.num if hasattr(sem, "num") else int(sem)
    ]
    last = None
    for n in nums:
        last = self.add_instruction(
            mybir.InstEventSemaphore(
                name=self.bass.get_next_instruction_name(),
                ins=[], outs=[],
                sync_info=mybir.SyncInfo(
                    on_wait=[],
                    on_update=[mybir.SyncUpdate(
                        sync_type="semaphore", id=n,
                        update_mode="sem-wr-imm", update_value=0)],
                ),
            )
        )
    return last


bass.BassGpSimd.sem_clear = _sem_clear_compat

_mw_ctr = [0]


def _split_multi_waits(nc: bass.Bass) -> None:
    for f in nc.m.functions:
        for bb in f.blocks:
            out = []
            changed = False
            for inst in bb.instructions:
                si = inst.sync_info
                waits = list(si.on_wait) if si is not None and si.on_wait else []
                if len(waits) > 1:
                    for w in waits[:-1]:
                        _mw_ctr[0] += 1
                        es = mybir.InstEventSemaphore(
                            name=f"I-mwsplit-{_mw_ctr[0]}",
                            engine=inst.engine,
                            ins=[], outs=[],
                            sync_info=mybir.SyncInfo(on_wait=[w], on_update=[]),
                        )
                        out.append(es)
                    inst.sync_info = mybir.SyncInfo(
                        on_wait=[waits[-1]],
                        on_update=list(si.on_update or []),
                    )
                    changed = True
                out.append(inst)
            if changed:
                bb.instructions = out
# -------------------------------------------------------------------------

B = 2048
D = 1024
N = 2 * B                 # 4096 z rows
NCORES = 8
TEMP = 0.05
INV_TEMP = 1.0 / TEMP
GC = 0.25                 # g2i exp shift: e^((sim-GC)/T) keeps fp8 escr in range
BPC = B // NCORES         # 256 image/text rows per core
ZPC = N // NCORES         # 512 z rows per core
NCH = D // 128            # 8 contraction chunks
NQ = NCH // 2             # 4 DoubleRow chunk-pairs
WIN = 2560                # g2i forward window per row-tile (5 banks of 512)
NJ = WIN // 512           # 5 g2i column banks
NT_G = ZPC // 128         # 4 g2i row-tiles per core
NT_L = BPC // 128         # 2 l2i row-tiles per core
NB_L = B // 512           # 4 l2i column banks

BF16 = mybir.dt.bfloat16
F32 = mybir.dt.float32
FP8 = mybir.dt.float8e4
AF = mybir.ActivationFunctionType
DR = mybir.MatmulPerfMode.DoubleRow

# stats column layout ([128, 16] f32 per core)
COL_LSE_IMG = 0   # +t (2): complete row lse (ls*gmax + ln S)
COL_POS_L2I = 2   # +t (2): raw positive dot (unscaled)
COL_FWD_G2I = 4   # +t (4): forward exp-sums, scaled by e^(-GC/T)
COL_POS_G2I = 8   # +t (4): positive-pair cosine sim

_cache: dict = {}


def _build_program(ls: float) -> bass.Bass:
    nc = bass.Bass(trn_type="TRN2")
    txt_d = nc.dram_tensor("txt", [D, B], BF16, kind="ExternalInput")
    # img is host-swizzled to partition-major [128, NCH*BPC] -> one DMA
    img_d = nc.dram_tensor("img", [128, NCH * BPC], BF16, kind="ExternalInput")
    z_d = nc.dram_tensor("z", [D, N], FP8, kind="ExternalInput")
    eye_d = nc.dram_tensor("eye", [128, 128], F32, kind="ExternalInput")
    ones_d = nc.dram_tensor("ones", [128, 256], FP8, kind="ExternalInput")
    mlo_d = nc.dram_tensor("mlo", [128, 4 * 512], BF16, kind="ExternalInput")
    mhi_d = nc.dram_tensor("mhi", [128, 4 * 512], BF16, kind="ExternalInput")
    stats_d = nc.dram_tensor("stats", [128, 16], F32, kind="ExternalOutput")
    l2m_d = nc.dram_tensor("l2m", [128, 2 * 16], F32, kind="ExternalOutput")
    g2c_d = nc.dram_tensor("g2c", [1, WIN], F32, kind="ExternalOutput")

    with TileContext(nc) as tc:
        with (
            tc.tile_pool(name="consts", bufs=1) as consts,
            tc.tile_pool(name="feat", bufs=1) as featp,
            tc.tile_pool(name="scr", bufs=2) as scrp,
            tc.tile_pool(name="escrp", bufs=4) as escrp,
            tc.tile_pool(name="stat", bufs=2) as statp,
            tc.tile_pool(name="mm", bufs=8, space="PSUM") as mmp,
        ):
            eye = consts.tile([128, 128], F32, tag="eye")
            ones = consts.tile([128, 2, 128], FP8, tag="ones")
            mlo = consts.tile([128, 4, 512], BF16, tag="mlo")
            mhi = consts.tile([128, 4, 512], BF16, tag="mhi")
            nc.gpsimd.dma_start(eye, eye_d[:, :])
            nc.gpsimd.dma_start(ones[:, :, :], ones_d[:, :])

            stats = consts.tile([128, 16], F32, tag="stats")
            l2m = consts.tile([128, 2 * 16], F32, tag="l2m")
            g2crow = consts.tile([1, WIN], F32, tag="g2crow")
            gcbias = consts.tile([128, 1], F32, tag="gcbias")
            nc.vector.memset(gcbias, -GC * INV_TEMP)

            # ---- input tiles ----
            txt = featp.tile([128, NCH, B], BF16, tag="txt")
            img = featp.tile([128, NCH, BPC], BF16, tag="img")
            z = featp.tile([128, NCH, N], FP8, tag="z")

            # Split DMA issue across engines: descriptor generation (~0.5-1us
            # per dma_start) serializes per issuing sequencer. txt/img on SP,
            # z on the Activation engine's queues, consts on gpsimd SWDGE.
            # First chunks in small pieces so phase A starts early.
            for p in range(8):
                cs = slice(p * 256, (p + 1) * 256)
                nc.sync.dma_start(txt[:, 0, cs], txt_d[0:128, cs])
            for p in range(4):
                cs = slice(p * 512, (p + 1) * 512)
                nc.sync.dma_start(img[:, 2 * p:2 * p + 2, :], img_d[:, cs])
            for p in range(4):
                cs = slice(p * 512, (p + 1) * 512)
                nc.sync.dma_start(txt[:, 1, cs], txt_d[128:256, cs])
            for c in range(2, NCH):
                r0 = c * 128
                for p in range(2):
                    cs = slice(p * 1024, (p + 1) * 1024)
                    nc.sync.dma_start(txt[:, c, cs], txt_d[r0:r0 + 128, cs])
            for c in range(NCH):
                r0 = c * 128
                for p in range(2):
                    cs = slice(p * 2048, (p + 1) * 2048)
                    nc.scalar.dma_start(z[:, c, cs], z_d[r0:r0 + 128, cs])
            # masks are needed only by the g2i phase (~60% in) -- load last
            nc.sync.dma_start(mlo[:, :, :], mlo_d[:, :])
            nc.sync.dma_start(mhi[:, :, :], mhi_d[:, :])

            # ================= phase A: l2i side-0 matmuls =================
            # two passes of 4 banks each: pass-0 chains complete as soon as
            # the last txt chunk lands, so the psum->sbuf copies (and the
            # scalar/DVE stats behind them) start ~12us earlier than with all
            # 8 chains open.
            psl = [[None] * NB_L for _ in range(NT_L)]
            cp = []
            for t in range(NT_L):
                cp.append(scrp.tile([128, B], F32, tag="cp", name="cpt",
                                    bufs=2))
            for half in range(2):
                for t in range(NT_L):
                    for b in (2 * half, 2 * half + 1):
                        psl[t][b] = mmp.tile([128, 512], F32, tag="ps",
                                             name="ps")
                for c in range(NCH):
                    for t in range(NT_L):
                        for b in (2 * half, 2 * half + 1):
                            nc.tensor.matmul(
                                psl[t][b],
                                img[:, c, t * 128:(t + 1) * 128],
                                txt[:, c, b * 512:(b + 1) * 512],
                                start=(c == 0), stop=(c == NCH - 1),
                            )
                for t in range(NT_L):
                    for b in (2 * half, 2 * half + 1):
                        nc.vector.tensor_copy(
                            cp[t][:, b * 512:(b + 1) * 512], psl[t][b])

            # ---- l2i side-0 row stats (emitted per g2i iteration, fills gaps)
            def emit_rowstats(t):
                scr = statp.tile([128, 128], F32, tag="posscr", name="scr")
                nc.gpsimd.tensor_mul(scr, cp[t][:, t * 128:(t + 1) * 128], eye)
                nc.vector.reduce_sum(
                    stats[:, COL_POS_L2I + t:COL_POS_L2I + t + 1],
                    scr, axis=mybir.AxisListType.X)
                maxs = statp.tile([128, NB_L], F32, tag="maxs", name="maxs")
                sums = statp.tile([128, NB_L], F32, tag="sums", name="sums")
                negmax = statp.tile([128, NB_L], F32, tag="negmax",
                                    name="negmax")
                for b in range(NB_L):
                    nc.vector.reduce_max(
                        maxs[:, b:b + 1], cp[t][:, b * 512:(b + 1) * 512],
                        axis=mybir.AxisListType.X)
                    nc.vector.tensor_scalar_mul(
                        negmax[:, b:b + 1], maxs[:, b:b + 1], -ls)
                    edump = scrp.tile([128, 512], BF16, tag="edump",
                                      name="edump", bufs=2)
                    nc.scalar.activation(
                        edump, cp[t][:, b * 512:(b + 1) * 512], AF.Exp,
                        bias=negmax[:, b:b + 1], scale=ls,
                        accum_out=sums[:, b:b + 1],
                    )
                gmax = statp.tile([128, 1], F32, tag="gmax", name="gmax")
                nc.vector.reduce_max(gmax, maxs, axis=mybir.AxisListType.X)
                neggmax = statp.tile([128, 1], F32, tag="neggmax",
                                     name="neggmax")
                nc.vector.tensor_scalar_mul(neggmax, gmax, -ls)
                w4 = statp.tile([128, NB_L], F32, tag="w4", name="w4")
                nc.scalar.activation(w4, maxs, AF.Exp, bias=neggmax, scale=ls)
                scr4 = statp.tile([128, NB_L], F32, tag="scr4", name="scr4")
                S = statp.tile([128, 1], F32, tag="S", name="S")
                nc.vector.tensor_mul(scr4, sums, w4)
                nc.vector.reduce_sum(S, scr4, axis=mybir.AxisListType.X)
                lnS = statp.tile([128, 1], F32, tag="lnS", name="lnS")
                nc.scalar.activation(lnS, S, AF.Ln)
                gms = statp.tile([128, 1], F32, tag="gms", name="gms")
                nc.vector.tensor_scalar_mul(gms, gmax, ls)
                nc.vector.tensor_add(
                    stats[:, COL_LSE_IMG + t:COL_LSE_IMG + t + 1], lnS, gms)

            # ================= g2i (symmetric, window-aligned) =================
            # z arrives pre-normalized (host folds 1/||z|| into the fp8 cast),
            # so psum = sim directly: exp reads PSUM, masks add on PSUM.
            gsum = []
            for t in range(NT_G):
                gsum.append(statp.tile([128, NJ], F32, tag="gsum",
                                       name="gsum", bufs=NT_G))

            # t-outer / q-outer / j-inner: the stationary lhsT (own-row block)
            # stays loaded across 5 consecutive matmuls instead of reloading
            # per matmul. Side-1 transpose groups interleave per t to fill
            # engine gaps.
            escr_tiles = {}  # (j, t//2) -> [128, 2, 512] fp8
            for t in range(NT_G):
                psg = []
                for j in range(NJ):
                    psg.append(mmp.tile([128, 512], F32, tag="ps", name="psg"))
                for q in range(NQ):
                    for j in range(NJ):
                        nc.tensor.matmul(
                            psg[j],
                            z[:, 2 * q:2 * q + 2, t * 128:(t + 1) * 128],
                            z[:, 2 * q:2 * q + 2, j * 512:(j + 1) * 512],
                            start=(q == 0), stop=(q == NQ - 1), perf_mode=DR,
                        )
                for j in range(NJ):
                    if j == NJ - 1:
                        # positive-pair sim: diag at cols t*128 (pre-mask)
                        pscr = statp.tile([128, 128], F32, tag="pscr",
                                          name="pscr")
                        nc.vector.tensor_mul(
                            pscr, psg[j][:, t * 128:(t + 1) * 128], eye)
                        nc.vector.reduce_sum(
                            stats[:, COL_POS_G2I + t:COL_POS_G2I + t + 1],
                            pscr, axis=mybir.AxisListType.X)
                        nc.vector.tensor_add(psg[j], psg[j], mhi[:, t, :])
                    if j == 0:
                        nc.vector.tensor_add(psg[j], psg[j], mlo[:, t, :])
                    key = (j, t // 2)
                    if key not in escr_tiles:
                        escr_tiles[key] = escrp.tile(
                            [128, 2, 512], FP8, tag="escr", name="escr",
                            bufs=10)
                    nc.scalar.activation(
                        escr_tiles[key][:, t % 2, :], psg[j], AF.Exp,
                        scale=INV_TEMP, bias=gcbias,
                        accum_out=gsum[t][:, j:j + 1],
                    )
                # ---- side-1 transposes for this t: 2 col-group tiles ----
                # Within a tile the 4 transposes chain (start only on first):
                # a start=True mid-bank write re-zeroes the whole 2KB region.
                for gp_ in (2 * t, 2 * t + 1):
                    tpg = mmp.tile([128, 512], F32, tag="ps", name="tpg")
                    for half in range(2):
                        g = 2 * gp_ + half
                        for tt in range(NT_L):
                            first = (half == 0 and tt == 0)
                            last = (half == 1 and tt == NT_L - 1)
                            nc.tensor.matmul(
                                tpg[:, half * 256 + tt * 128:
                                    half * 256 + (tt + 1) * 128],
                                cp[tt][:, g * 128:(g + 1) * 128], eye,
                                is_transpose=True, start=first, stop=last,
                                skip_group_check=True,
                            )
                    for half in range(2):
                        g = 2 * gp_ + half
                        sl = slice(half * 256, (half + 1) * 256)
                        nc.vector.reduce_max(
                            l2m[:, g:g + 1], tpg[:, sl],
                            axis=mybir.AxisListType.X)
                        negm = statp.tile([128, 1], F32, tag="negm",
                                          name="negm")
                        nc.vector.tensor_scalar_mul(negm, l2m[:, g:g + 1], -ls)
                        edump2 = scrp.tile([128, 256], BF16, tag="edump2",
                                           name="edump2", bufs=2)
                        nc.scalar.activation(
                            edump2, tpg[:, sl], AF.Exp, bias=negm, scale=ls,
                            accum_out=l2m[:, 16 + g:16 + g + 1])
                if 1 <= t <= NT_L:
                    emit_rowstats(t - 1)

            # column sums over all 512 rows: 2 DR ones-matmuls per bank
            for j in range(NJ):
                cps = mmp.tile([128, 512], F32, tag="ps", name="cps")
                for h in range(2):
                    nc.tensor.matmul(
                        cps, ones[:, :, :], escr_tiles[(j, h)][:, :, :],
                        start=(h == 0), stop=(h == 1), perf_mode=DR,
                    )
                nc.vector.tensor_copy(g2crow[:, j * 512:(j + 1) * 512],
                                      cps[0:1, :])

            for t in range(NT_G):
                nc.vector.reduce_sum(
                    stats[:, COL_FWD_G2I + t:COL_FWD_G2I + t + 1],
                    gsum[t], axis=mybir.AxisListType.X)

            nc.sync.dma_start(stats_d[:, :], stats)
            nc.sync.dma_start(l2m_d[:, :], l2m)
            nc.sync.dma_start(g2c_d[:, :], g2crow)

    _split_multi_waits(nc)
    return nc


def _get_program(ls: float) -> bass.Bass:
    key = float(ls)
    if key not in _cache:
        _cache[key] = _build_program(key)
    return _cache[key]


def _make_masks() -> tuple[np.ndarray, np.ndarray]:
    """mlo[t]: -1e30 where window-col w <= 128t+p (backward + self, bank 0).
    mhi[t]: -1e30 where w-2048 >= 128t+p (beyond-window + pos col, bank 4)."""
    bf = ml_dtypes.bfloat16
    p = np.arange(128)[:, None]
    w = np.arange(512)[None, :]
    mlo = np.zeros((128, 4 * 512), dtype=np.float32)
    mhi = np.zeros((128, 4 * 512), dtype=np.float32)
    for t in range(4):
        lr = 128 * t + p
        mlo[:, t * 512:(t + 1) * 512] = np.where(w <= lr, -1e30, 0.0)
        mhi[:, t * 512:(t + 1) * 512] = np.where(w >= lr, -1e30, 0.0)
    return mlo.astype(bf), mhi.astype(bf)


def kernel(image_features, gli_features, text_features, logit_scale):
    ls = float(np.asarray(logit_scale))
    nc = _get_program(ls)

    bf = ml_dtypes.bfloat16
    f8 = ml_dtypes.float8_e4m3fn
    imgT = np.ascontiguousarray(np.asarray(image_features, dtype=np.float32).T)
    txtT = np.ascontiguousarray(np.asarray(text_features, dtype=np.float32).T)
    zT = np.ascontiguousarray(np.concatenate(
        [np.asarray(gli_features, dtype=np.float32),
         np.asarray(image_features, dtype=np.float32)], axis=0).T)
    # fold 1/||z_fp8|| into the fp8 sharding cast: the device GEMM then
    # produces cosine similarities directly
    z8f = zT.astype(f8).astype(np.float32)
    zn8 = (z8f / np.sqrt((z8f * z8f).sum(0, keepdims=True))).astype(f8)

    eye = np.eye(128, dtype=np.float32)
    ones = np.ones((128, 256), dtype=f8)
    mlo, mhi = _make_masks()

    in_maps = []
    for k in range(NCORES):
        txt_k = np.roll(txtT, -BPC * k, axis=1).astype(bf)
        # img swizzled partition-major: [128, NCH*BPC]
        img_k = (imgT[:, BPC * k:BPC * (k + 1)].astype(bf)
                 .reshape(NCH, 128, BPC).transpose(1, 0, 2).reshape(128, -1))
        z_k = np.roll(zn8, -ZPC * k, axis=1)
        in_maps.append({
            "txt": np.ascontiguousarray(txt_k),
            "img": np.ascontiguousarray(img_k),
            "z": np.ascontiguousarray(z_k),
            "eye": eye, "ones": ones, "mlo": mlo, "mhi": mhi,
        })

    res = bass_utils.run_bass_kernel_spmd(nc, in_maps, core_ids=list(range(NCORES)))
    globals()["LAST_RESULT"] = res

    stats = np.stack([r["stats"] for r in res.results]).astype(np.float64)
    l2m = np.stack([r["l2m"] for r in res.results]).astype(np.float64)
    g2c = np.stack([r["g2c"] for r in res.results]).astype(np.float64)[:, 0, :]

    # ---- l2i ----
    lse_img = stats[:, :, COL_LSE_IMG:COL_LSE_IMG + NT_L]  # [8,128,2]
    pos_l2i = stats[:, :, COL_POS_L2I:COL_POS_L2I + NT_L]
    sum_lse_img = lse_img.sum()
    sum_pos = pos_l2i.sum()

    # side-1: combine per-core column partials. core k local col g*128+p is
    # global col (256k + g*128 + p) mod 2048; partial (m, S) covers 256 rows.
    m_part = l2m[:, :, 0:16]    # [8, 128, 16]
    s_part = l2m[:, :, 16:32]
    cols = (np.arange(NCORES)[:, None, None] * BPC
            + np.arange(16)[None, None, :] * 128
            + np.arange(128)[None, :, None]) % B  # [8,128,16]
    m_all = np.zeros((NCORES, B))
    s_all = np.zeros((NCORES, B))
    for k in range(NCORES):
        m_all[k, cols[k].ravel()] = m_part[k].ravel()
        s_all[k, cols[k].ravel()] = s_part[k].ravel()
    m_glob = m_all.max(0)
    lse_txt = ls * m_glob + np.log(
        (s_all * np.exp(ls * (m_all - m_glob[None, :]))).sum(0))
    sum_lse_txt = lse_txt.sum()

    l2i = 0.5 * ((sum_lse_img - ls * sum_pos) / B
                 + (sum_lse_txt - ls * sum_pos) / B)

    # ---- g2i ----
    fwd = stats[:, :, COL_FWD_G2I:COL_FWD_G2I + NT_G]   # [8,128,4] scaled sums
    pos_g2i = stats[:, :, COL_POS_G2I:COL_POS_G2I + NT_G]
    fwd_rows = np.zeros((N,))
    pos_rows = np.zeros((N,))
    rows = (np.arange(NCORES)[:, None, None] * ZPC
            + np.arange(NT_G)[None, None, :] * 128
            + np.arange(128)[None, :, None])  # [8,128,4]
    for k in range(NCORES):
        fwd_rows[rows[k].ravel()] = fwd[k].ravel()
        pos_rows[rows[k].ravel()] = pos_g2i[k].ravel()
    back_rows = np.zeros((N,))
    for k in range(NCORES):
        gcols = (np.arange(WIN) + ZPC * k) % N
        np.add.at(back_rows, gcols, g2c[k])
    # fwd+back are scaled by e^(-GC/T); add pos term and unshift in the log
    total = fwd_rows + back_rows + np.exp((pos_rows - GC) * INV_TEMP)
    lse = np.log(total) + GC * INV_TEMP
    g2i = (lse - pos_rows * INV_TEMP).sum() / N

    tot = l2i + g2i
    return (np.float32(tot), np.float32(l2i), np.float32(g2i))


# revision 36
# speedup vs baseline: 1.0870x; 1.0870x over previous
"""Distributed CLIP-style loss (l2i symmetric CE + g2i NT-Xent) on 8 TRN2 cores.

Strategy (v2): data-parallel row sharding with column-rotated per-core inputs
(rotation = the core's global row offset) so diagonals sit at static local
offsets and one SPMD program serves all 8 cores.

Key optimizations over v1:
 - z ships as fp8-e4m3 (4MB/core instead of 8MB bf16); the g2i GEMM runs in
   fp8 DoubleRow mode (2x PE throughput). Norms are computed on-device from
   fp8 squares via a DoubleRow ones-matmul.
 - g2i exploits symmetry of the similarity matrix: each 128-row tile computes
   only the column window [0, 2560) (its forward half, window-aligned across
   tiles); the backward half comes from column sums of the exp'd forward
   blocks (fp8 ones-matmul), assembled with the row sums on the host.
 - l2i computes the image@text GEMM once. Row-side LSE is computed on-device;
   the text-side (column) LSE partials come from f32 PE-transposes of the
   logits + per-column (max, sum) over the core's 256 rows, combined on host.
 - norm pipeline (fp8 squares -> ones-mm -> recip -> sqrt) is split across
   DVE/gpsimd and interleaved with phase A.
"""

import numpy as np
import ml_dtypes

import concourse.bass as bass
import concourse.mybir as mybir
from concourse.tile import TileContext
from concourse import bass_utils


# --- compat patches for the walrus build in this container ---------------
def _sem_clear_compat(self, sem):
    nums = list(sem) if isinstance(sem, range) else [
        sem.num if hasattr(sem, "num") else int(sem)
    ]
    last = None
    for n in nums:
        last = self.add_instruction(
            mybir.InstEventSemaphore(
                name=self.bass.get_next_instruction_name(),
                ins=[], outs=[],
                sync_info=mybir.SyncInfo(
                    on_wait=[],
                    on_update=[mybir.SyncUpdate(
                        sync_type="semaphore", id=n,
                        update_mode="sem-wr-imm", update_value=0)],
                ),
            )
        )
    return last


bass.BassGpSimd.sem_clear = _sem_clear_compat

_mw_ctr = [0]


def _split_multi_waits(nc: bass.Bass) -> None:
    for f in nc.m.functions:
        for bb in f.blocks:
            out = []
            changed = False
            for inst in bb.instructions:
                si = inst.sync_info
                waits = list(si.on_wait) if si is not None and si.on_wait else []
                if len(waits) > 1:
                    for w in waits[:-1]:
                        _mw_ctr[0] += 1
                        es = mybir.InstEventSemaphore(
                            name=f"I-mwsplit-{_mw_ctr[0]}",
                            engine=inst.engine,
                            ins=[], outs=[],
                            sync_info=mybir.SyncInfo(on_wait=[w], on_update=[]),
                        )
                        out.append(es)
                    inst.sync_info = mybir.SyncInfo(
                        on_wait=[waits[-1]],
                        on_update=list(si.on_update or []),
                    )
                    changed = True
                out.append(inst)
            if changed:
                bb.instructions = out
# -------------------------------------------------------------------------

B = 2048
D = 1024
N = 2 * B                 # 4096 z rows
NCORES = 8
TEMP = 0.05
INV_TEMP = 1.0 / TEMP
GC = 0.25                 # g2i exp shift: e^((sim-GC)/T) keeps fp8 escr in range
BPC = B // NCORES         # 256 image/text rows per core
ZPC = N // NCORES         # 512 z rows per core
NCH = D // 128            # 8 contraction chunks
NQ = NCH // 2             # 4 DoubleRow chunk-pairs
WIN = 2560                # g2i forward window per row-tile (5 banks of 512)
NJ = WIN // 512           # 5 g2i column banks
NT_G = ZPC // 128         # 4 g2i row-tiles per core
NT_L = BPC // 128         # 2 l2i row-tiles per core
NB_L = B // 512           # 4 l2i column banks

BF16 = mybir.dt.bfloat16
F32 = mybir.dt.float32
FP8 = mybir.dt.float8e4
AF = mybir.ActivationFunctionType
DR = mybir.MatmulPerfMode.DoubleRow

# stats column layout ([128, 16] f32 per core)
COL_LSE_IMG = 0   # +t (2): complete row lse (ls*gmax + ln S)
COL_POS_L2I = 2   # +t (2): raw positive dot (unscaled)
COL_FWD_G2I = 4   # +t (4): forward exp-sums, scaled by e^(-GC/T)
COL_POS_G2I = 8   # +t (4): positive-pair cosine sim

_cache: dict = {}


def _build_program(ls: float) -> bass.Bass:
    nc = bass.Bass(trn_type="TRN2")
    txt_d = nc.dram_tensor("txt", [D, B], BF16, kind="ExternalInput")
    # img is host-swizzled to partition-major [128, NCH*BPC] -> one DMA
    img_d = nc.dram_tensor("img", [128, NCH * BPC], BF16, kind="ExternalInput")
    z_d = nc.dram_tensor("z", [D, N], FP8, kind="ExternalInput")
    eye_d = nc.dram_tensor("eye", [128, 128], F32, kind="ExternalInput")
    ones_d = nc.dram_tensor("ones", [128, 256], FP8, kind="ExternalInput")
    mlo_d = nc.dram_tensor("mlo", [128, 4 * 512], BF16, kind="ExternalInput")
    mhi_d = nc.dram_tensor("mhi", [128, 4 * 512], BF16, kind="ExternalInput")
    stats_d = nc.dram_tensor("stats", [128, 16], F32, kind="ExternalOutput")
    l2m_d = nc.dram_tensor("l2m", [128, 2 * 16], F32, kind="ExternalOutput")
    g2c_d = nc.dram_tensor("g2c", [1, WIN], F32, kind="ExternalOutput")

    with TileContext(nc) as tc:
        with (
            tc.tile_pool(name="consts", bufs=1) as consts,
            tc.tile_pool(name="feat", bufs=1) as featp,
            tc.tile_pool(name="scr", bufs=2) as scrp,
            tc.tile_pool(name="escrp", bufs=4) as escrp,
            tc.tile_pool(name="stat", bufs=2) as statp,
            tc.tile_pool(name="mm", bufs=8, space="PSUM") as mmp,
        ):
            eye = consts.tile([128, 128], F32, tag="eye")
            ones = consts.tile([128, 2, 128], FP8, tag="ones")
            mlo = consts.tile([128, 4, 512], BF16, tag="mlo")
            mhi = consts.tile([128, 4, 512], BF16, tag="mhi")
            nc.gpsimd.dma_start(eye, eye_d[:, :])
            nc.gpsimd.dma_start(ones[:, :, :], ones_d[:, :])

            stats = consts.tile([128, 16], F32, tag="stats")
            l2m = consts.tile([128, 2 * 16], F32, tag="l2m")
            g2crow = consts.tile([1, WIN], F32, tag="g2crow")
            gcbias = consts.tile([128, 1], F32, tag="gcbias")
            nc.vector.memset(gcbias, -GC * INV_TEMP)

            # ---- input tiles ----
            txt = featp.tile([128, NCH, B], BF16, tag="txt")
            img = featp.tile([128, NCH, BPC], BF16, tag="img")
            z = featp.tile([128, NCH, N], FP8, tag="z")

            # Split DMA issue across engines: descriptor generation (~0.5-1us
            # per dma_start) serializes per issuing sequencer. txt/img on SP,
            # z on the Activation engine's queues, consts on gpsimd SWDGE.
            # First chunks in small pieces so phase A starts early.
            for p in range(4):
                cs = slice(p * 512, (p + 1) * 512)
                nc.sync.dma_start(txt[:, 0, cs], txt_d[0:128, cs])
            for p in range(4):
                cs = slice(p * 512, (p + 1) * 512)
                nc.sync.dma_start(img[:, 2 * p:2 * p + 2, :], img_d[:, cs])
            for p in range(4):
                cs = slice(p * 512, (p + 1) * 512)
                nc.sync.dma_start(txt[:, 1, cs], txt_d[128:256, cs])
            for c in range(2, NCH):
                r0 = c * 128
                for p in range(2):
                    cs = slice(p * 1024, (p + 1) * 1024)
                    nc.sync.dma_start(txt[:, c, cs], txt_d[r0:r0 + 128, cs])
            for c in range(NCH):
                r0 = c * 128
                for p in range(2):
                    cs = slice(p * 2048, (p + 1) * 2048)
                    nc.scalar.dma_start(z[:, c, cs], z_d[r0:r0 + 128, cs])
            # masks are needed only by the g2i phase (~60% in) -- load last
            nc.sync.dma_start(mlo[:, :, :], mlo_d[:, :])
            nc.sync.dma_start(mhi[:, :, :], mhi_d[:, :])

            # ================= phase A: l2i side-0 matmuls =================
            # two passes of 4 banks each: pass-0 chains complete as soon as
            # the last txt chunk lands, so the psum->sbuf copies (and the
            # scalar/DVE stats behind them) start ~12us earlier than with all
            # 8 chains open.
            psl = [[None] * NB_L for _ in range(NT_L)]
            cp = []
            for t in range(NT_L):
                cp.append(scrp.tile([128, B], F32, tag="cp", name="cpt",
                                    bufs=2))
            for half in range(2):
                for t in range(NT_L):
                    for b in (2 * half, 2 * half + 1):
                        psl[t][b] = mmp.tile([128, 512], F32, tag="ps",
                                             name="ps")
                for c in range(NCH):
                    for t in range(NT_L):
                        for b in (2 * half, 2 * half + 1):
                            nc.tensor.matmul(
                                psl[t][b],
                                img[:, c, t * 128:(t + 1) * 128],
                                txt[:, c, b * 512:(b + 1) * 512],
                                start=(c == 0), stop=(c == NCH - 1),
                            )
                for t in range(NT_L):
                    for b in (2 * half, 2 * half + 1):
                        nc.vector.tensor_copy(
                            cp[t][:, b * 512:(b + 1) * 512], psl[t][b])

            # ---- l2i side-0 row stats (emitted per g2i iteration, fills gaps)
            def emit_rowstats(t):
                scr = statp.tile([128, 128], F32, tag="posscr", name="scr")
                nc.gpsimd.tensor_mul(scr, cp[t][:, t * 128:(t + 1) * 128], eye)
                nc.vector.reduce_sum(
                    stats[:, COL_POS_L2I + t:COL_POS_L2I + t + 1],
                    scr, axis=mybir.AxisListType.X)
                maxs = statp.tile([128, NB_L], F32, tag="maxs", name="maxs")
                sums = statp.tile([128, NB_L], F32, tag="sums", name="sums")
                negmax = statp.tile([128, NB_L], F32, tag="negmax",
                                    name="negmax")
                for b in range(NB_L):
                    nc.vector.reduce_max(
                        maxs[:, b:b + 1], cp[t][:, b * 512:(b + 1) * 512],
                        axis=mybir.AxisListType.X)
                    nc.vector.tensor_scalar_mul(
                        negmax[:, b:b + 1], maxs[:, b:b + 1], -ls)
                    edump = scrp.tile([128, 512], BF16, tag="edump",
                                      name="edump", bufs=2)
                    nc.scalar.activation(
                        edump, cp[t][:, b * 512:(b + 1) * 512], AF.Exp,
                        bias=negmax[:, b:b + 1], scale=ls,
                        accum_out=sums[:, b:b + 1],
                    )
                gmax = statp.tile([128, 1], F32, tag="gmax", name="gmax")
                nc.vector.reduce_max(gmax, maxs, axis=mybir.AxisListType.X)
                neggmax = statp.tile([128, 1], F32, tag="neggmax",
                                     name="neggmax")
                nc.vector.tensor_scalar_mul(neggmax, gmax, -ls)
                w4 = statp.tile([128, NB_L], F32, tag="w4", name="w4")
                nc.scalar.activation(w4, maxs, AF.Exp, bias=neggmax, scale=ls)
                scr4 = statp.tile([128, NB_L], F32, tag="scr4", name="scr4")
                S = statp.tile([128, 1], F32, tag="S", name="S")
                nc.vector.tensor_mul(scr4, sums, w4)
                nc.vector.reduce_sum(S, scr4, axis=mybir.AxisListType.X)
                lnS = statp.tile([128, 1], F32, tag="lnS", name="lnS")
                nc.scalar.activation(lnS, S, AF.Ln)
                gms = statp.tile([128, 1], F32, tag="gms", name="gms")
                nc.vector.tensor_scalar_mul(gms, gmax, ls)
                nc.vector.tensor_add(
                    stats[:, COL_LSE_IMG + t:COL_LSE_IMG + t + 1], lnS, gms)

            # ================= g2i (symmetric, window-aligned) =================
            # z arrives pre-normalized (host folds 1/||z|| into the fp8 cast),
            # so psum = sim directly: exp reads PSUM, masks add on PSUM.
            gsum = []
            for t in range(NT_G):
                gsum.append(statp.tile([128, NJ], F32, tag="gsum",
                                       name="gsum", bufs=NT_G))

            # t-outer / q-outer / j-inner: the stationary lhsT (own-row block)
            # stays loaded across 5 consecutive matmuls instead of reloading
            # per matmul. Side-1 transpose groups interleave per t to fill
            # engine gaps.
            escr_tiles = {}  # (j, t//2) -> [128, 2, 512] fp8
            for t in range(NT_G):
                psg = []
                for j in range(NJ):
                    psg.append(mmp.tile([128, 512], F32, tag="ps", name="psg"))
                for q in range(NQ):
                    for j in range(NJ):
                        nc.tensor.matmul(
                            psg[j],
                            z[:, 2 * q:2 * q + 2, t * 128:(t + 1) * 128],
                            z[:, 2 * q:2 * q + 2, j * 512:(j + 1) * 512],
                            start=(q == 0), stop=(q == NQ - 1), perf_mode=DR,
                        )
                for j in range(NJ):
                    if j == NJ - 1:
                        # positive-pair sim: diag at cols t*128 (pre-mask)
                        pscr = statp.tile([128, 128], F32, tag="pscr",
                                          name="pscr")
                        nc.vector.tensor_mul(
                            pscr, psg[j][:, t * 128:(t + 1) * 128], eye)
                        nc.vector.reduce_sum(
                            stats[:, COL_POS_G2I + t:COL_POS_G2I + t + 1],
                            pscr, axis=mybir.AxisListType.X)
                        nc.vector.tensor_add(psg[j], psg[j], mhi[:, t, :])
                    if j == 0:
                        nc.vector.tensor_add(psg[j], psg[j], mlo[:, t, :])
                    key = (j, t // 2)
                    if key not in escr_tiles:
                        escr_tiles[key] = escrp.tile(
                            [128, 2, 512], FP8, tag="escr", name="escr",
                            bufs=10)
                    nc.scalar.activation(
                        escr_tiles[key][:, t % 2, :], psg[j], AF.Exp,
                        scale=INV_TEMP, bias=gcbias,
                        accum_out=gsum[t][:, j:j + 1],
                    )
                # ---- side-1 transposes for this t: 2 col-group tiles ----
                # Within a tile the 4 transposes chain (start only on first):
                # a start=True mid-bank write re-zeroes the whole 2KB region.
                for gp_ in (2 * t, 2 * t + 1):
                    tpg = mmp.tile([128, 512], F32, tag="ps", name="tpg")
                    for half in range(2):
                        g = 2 * gp_ + half
                        for tt in range(NT_L):
                            first = (half == 0 and tt == 0)
                            last = (half == 1 and tt == NT_L - 1)
                            nc.tensor.matmul(
                                tpg[:, half * 256 + tt * 128:
                                    half * 256 + (tt + 1) * 128],
                                cp[tt][:, g * 128:(g + 1) * 128], eye,
                                is_transpose=True, start=first, stop=last,
                                skip_group_check=True,
                            )
                    for half in range(2):
                        g = 2 * gp_ + half
                        sl = slice(half * 256, (half + 1) * 256)
                        nc.vector.reduce_max(
                            l2m[:, g:g + 1], tpg[:, sl],
                            axis=mybir.AxisListType.X)
                        negm = statp.tile([128, 1], F32, tag="negm",
                                          name="negm")
                        nc.vector.tensor_scalar_mul(negm, l2m[:, g:g + 1], -ls)
                        edump2 = scrp.tile([128, 256], BF16, tag="edump2",
                                           name="edump2", bufs=2)
                        nc.scalar.activation(
                            edump2, tpg[:, sl], AF.Exp, bias=negm, scale=ls,
                            accum_out=l2m[:, 16 + g:16 + g + 1])
                if t >= NT_G - NT_L:
                    emit_rowstats(t - (NT_G - NT_L))

            # column sums over all 512 rows: 2 DR ones-matmuls per bank
            for j in range(NJ):
                cps = mmp.tile([128, 512], F32, tag="ps", name="cps")
                for h in range(2):
                    nc.tensor.matmul(
                        cps, ones[:, :, :], escr_tiles[(j, h)][:, :, :],
                        start=(h == 0), stop=(h == 1), perf_mode=DR,
                    )
                nc.vector.tensor_copy(g2crow[:, j * 512:(j + 1) * 512],
                                      cps[0:1, :])

            for t in range(NT_G):
                nc.vector.reduce_sum(
                    stats[:, COL_FWD_G2I + t:COL_FWD_G2I + t + 1],
                    gsum[t], axis=mybir.AxisListType.X)

            nc.sync.dma_start(stats_d[:, :], stats)
            nc.sync.dma_start(l2m_d[:, :], l2m)
            nc.sync.dma_start(g2c_d[:, :], g2crow)

    _split_multi_waits(nc)
    return nc


def _get_program(ls: float) -> bass.Bass:
    key = float(ls)
    if key not in _cache:
        _cache[key] = _build_program(key)
    return _cache[key]


def _make_masks() -> tuple[np.ndarray, np.ndarray]:
    """mlo[t]: -1e30 where window-col w <= 128t+p (backward + self, bank 0).
    mhi[t]: -1e30 where w-2048 >= 128t+p (beyond-window + pos col, bank 4)."""
    bf = ml_dtypes.bfloat16
    p = np.arange(128)[:, None]
    w = np.arange(512)[None, :]
    mlo = np.zeros((128, 4 * 512), dtype=np.float32)
    mhi = np.zeros((128, 4 * 512), dtype=np.float32)
    for t in range(4):
        lr = 128 * t + p
        mlo[:, t * 512:(t + 1) * 512] = np.where(w <= lr, -1e30, 0.0)
        mhi[:, t * 512:(t + 1) * 512] = np.where(w >= lr, -1e30, 0.0)
    return mlo.astype(bf), mhi.astype(bf)


def kernel(image_features, gli_features, text_features, logit_scale):
    ls = float(np.asarray(logit_scale))
    nc = _get_program(ls)

    bf = ml_dtypes.bfloat16
    f8 = ml_dtypes.float8_e4m3fn
    imgT = np.ascontiguousarray(np.asarray(image_features, dtype=np.float32).T)
    txtT = np.ascontiguousarray(np.asarray(text_features, dtype=np.float32).T)
    zT = np.ascontiguousarray(np.concatenate(
        [np.asarray(gli_features, dtype=np.float32),
         np.asarray(image_features, dtype=np.float32)], axis=0).T)
    # fold 1/||z_fp8|| into the fp8 sharding cast: the device GEMM then
    # produces cosine similarities directly
    z8f = zT.astype(f8).astype(np.float32)
    zn8 = (z8f / np.sqrt((z8f * z8f).sum(0, keepdims=True))).astype(f8)

    eye = np.eye(128, dtype=np.float32)
    ones = np.ones((128, 256), dtype=f8)
    mlo, mhi = _make_masks()

    in_maps = []
    for k in range(NCORES):
        txt_k = np.roll(txtT, -BPC * k, axis=1).astype(bf)
        # img swizzled partition-major: [128, NCH*BPC]
        img_k = (imgT[:, BPC * k:BPC * (k + 1)].astype(bf)
                 .reshape(NCH, 128, BPC).transpose(1, 0, 2).reshape(128, -1))
        z_k = np.roll(zn8, -ZPC * k, axis=1)
        in_maps.append({
            "txt": np.ascontiguousarray(txt_k),
            "img": np.ascontiguousarray(img_k),
            "z": np.ascontiguousarray(z_k),
            "eye": eye, "ones": ones, "mlo": mlo, "mhi": mhi,
        })

    res = bass_utils.run_bass_kernel_spmd(nc, in_maps, core_ids=list(range(NCORES)))
    globals()["LAST_RESULT"] = res

    stats = np.stack([r["stats"] for r in res.results]).astype(np.float64)
    l2m = np.stack([r["l2m"] for r in res.results]).astype(np.float64)
    g2c = np.stack([r["g2c"] for r in res.results]).astype(np.float64)[:, 0, :]

    # ---- l2i ----
    lse_img = stats[:, :, COL_LSE_IMG:COL_LSE_IMG + NT_L]  # [8,128,2]
    pos_l2i = stats[:, :, COL_POS_L2I:COL_POS_L2I + NT_L]
    sum_lse_img = lse_img.sum()
    sum_pos = pos_l2i.sum()

    # side-1: combine per-core column partials. core k local col g*128+p is
    # global col (256k + g*128 + p) mod 2048; partial (m, S) covers 256 rows.
    m_part = l2m[:, :, 0:16]    # [8, 128, 16]
    s_part = l2m[:, :, 16:32]
    cols = (np.arange(NCORES)[:, None, None] * BPC
            + np.arange(16)[None, None, :] * 128
            + np.arange(128)[None, :, None]) % B  # [8,128,16]
    m_all = np.zeros((NCORES, B))
    s_all = np.zeros((NCORES, B))
    for k in range(NCORES):
        m_all[k, cols[k].ravel()] = m_part[k].ravel()
        s_all[k, cols[k].ravel()] = s_part[k].ravel()
    m_glob = m_all.max(0)
    lse_txt = ls * m_glob + np.log(
        (s_all * np.exp(ls * (m_all - m_glob[None, :]))).sum(0))
    sum_lse_txt = lse_txt.sum()

    l2i = 0.5 * ((sum_lse_img - ls * sum_pos) / B
                 + (sum_lse_txt - ls * sum_pos) / B)

    # ---- g2i ----
    fwd = stats[:, :, COL_FWD_G2I:COL_FWD_G2I + NT_G]   # [8,128,4] scaled sums
    pos_g2i = stats[:, :, COL_POS_G2I:COL_POS_G2I + NT_G]
    fwd_rows = np.zeros((N,))
    pos_rows = np.zeros((N,))
    rows = (np.arange(NCORES)[:, None, None] * ZPC
            + np.arange(NT_G)[None, None, :] * 128
            + np.arange(128)[None, :, None])  # [8,128,4]
    for k in range(NCORES):
        fwd_rows[rows[k].ravel()] = fwd[k].ravel()
        pos_rows[rows[k].ravel()] = pos_g2i[k].ravel()
    back_rows = np.zeros((N,))
    for k in range(NCORES):
        gcols = (np.arange(WIN) + ZPC * k) % N
        np.add.at(back_rows, gcols, g2c[k])
    # fwd+back are scaled by e^(-GC/T); add pos term and unshift in the log
    total = fwd_rows + back_rows + np.exp((pos_rows - GC) * INV_TEMP)
    lse = np.log(total) + GC * INV_TEMP
    g2i = (lse - pos_rows * INV_TEMP).sum() / N

    tot = l2i + g2i
    return (np.float32(tot), np.float32(l2i), np.float32(g2i))


# revision 38
# speedup vs baseline: 1.1122x; 1.0232x over previous
"""Distributed CLIP-style loss (l2i symmetric CE + g2i NT-Xent) on 8 TRN2 cores.

Strategy (v2): data-parallel row sharding with column-rotated per-core inputs
(rotation = the core's global row offset) so diagonals sit at static local
offsets and one SPMD program serves all 8 cores.

Key optimizations over v1:
 - z ships as fp8-e4m3 (4MB/core instead of 8MB bf16); the g2i GEMM runs in
   fp8 DoubleRow mode (2x PE throughput). Norms are computed on-device from
   fp8 squares via a DoubleRow ones-matmul.
 - g2i exploits symmetry of the similarity matrix: each 128-row tile computes
   only the column window [0, 2560) (its forward half, window-aligned across
   tiles); the backward half comes from column sums of the exp'd forward
   blocks (fp8 ones-matmul), assembled with the row sums on the host.
 - l2i computes the image@text GEMM once. Row-side LSE is computed on-device;
   the text-side (column) LSE partials come from f32 PE-transposes of the
   logits + per-column (max, sum) over the core's 256 rows, combined on host.
 - norm pipeline (fp8 squares -> ones-mm -> recip -> sqrt) is split across
   DVE/gpsimd and interleaved with phase A.
"""

import numpy as np
import ml_dtypes

import concourse.bass as bass
import concourse.mybir as mybir
from concourse.tile import TileContext
from concourse import bass_utils


# --- compat patches for the walrus build in this container ---------------
def _sem_clear_compat(self, sem):
    nums = list(sem) if isinstance(sem, range) else [
        sem.num if hasattr(sem, "num") else int(sem)
    ]
    last = None
    for n in nums:
        last = self.add_instruction(
            mybir.InstEventSemaphore(
                name=self.bass.get_next_instruction_name(),
                ins=[], outs=[],
                sync_info=mybir.SyncInfo(
                    on_wait=[],
                    on_update=[mybir.SyncUpdate(
                        sync_type="semaphore", id=n,
                        update_mode="sem-wr-imm", update_value=0)],
                ),
            )
        )
    return last


bass.BassGpSimd.sem_clear = _sem_clear_compat

_mw_ctr = [0]


def _split_multi_waits(nc: bass.Bass) -> None:
    for f in nc.m.functions:
        for bb in f.blocks:
            out = []
            changed = False
            for inst in bb.instructions:
                si = inst.sync_info
                waits = list(si.on_wait) if si is not None and si.on_wait else []
                if len(waits) > 1:
                    for w in waits[:-1]:
                        _mw_ctr[0] += 1
                        es = mybir.InstEventSemaphore(
                            name=f"I-mwsplit-{_mw_ctr[0]}",
                            engine=inst.engine,
                            ins=[], outs=[],
                            sync_info=mybir.SyncInfo(on_wait=[w], on_update=[]),
                        )
                        out.append(es)
                    inst.sync_info = mybir.SyncInfo(
                        on_wait=[waits[-1]],
                        on_update=list(si.on_update or []),
                    )
                    changed = True
                out.append(inst)
            if changed:
                bb.instructions = out
# -------------------------------------------------------------------------

B = 2048
D = 1024
N = 2 * B                 # 4096 z rows
NCORES = 8
TEMP = 0.05
INV_TEMP = 1.0 / TEMP
GC = 0.25                 # g2i exp shift: e^((sim-GC)/T) keeps fp8 escr in range
BPC = B // NCORES         # 256 image/text rows per core
ZPC = N // NCORES         # 512 z rows per core
NCH = D // 128            # 8 contraction chunks
NQ = NCH // 2             # 4 DoubleRow chunk-pairs
WIN = 2560                # g2i forward window per row-tile (5 banks of 512)
NJ = WIN // 512           # 5 g2i column banks
NT_G = ZPC // 128         # 4 g2i row-tiles per core
NT_L = BPC // 128         # 2 l2i row-tiles per core
NB_L = B // 512           # 4 l2i column banks

BF16 = mybir.dt.bfloat16
F32 = mybir.dt.float32
FP8 = mybir.dt.float8e4
AF = mybir.ActivationFunctionType
DR = mybir.MatmulPerfMode.DoubleRow

# stats column layout ([128, 16] f32 per core)
COL_LSE_IMG = 0   # +t (2): complete row lse (ls*gmax + ln S)
COL_POS_L2I = 2   # +t (2): raw positive dot (unscaled)
COL_FWD_G2I = 4   # +t (4): forward exp-sums, scaled by e^(-GC/T)
COL_POS_G2I = 8   # +t (4): positive-pair cosine sim

_cache: dict = {}


def _build_program(ls: float) -> bass.Bass:
    nc = bass.Bass(trn_type="TRN2")
    txt_d = nc.dram_tensor("txt", [D, B], BF16, kind="ExternalInput")
    # img is host-swizzled to partition-major [128, NCH*BPC] -> one DMA
    img_d = nc.dram_tensor("img", [128, NCH * BPC], BF16, kind="ExternalInput")
    z_d = nc.dram_tensor("z", [D, N], FP8, kind="ExternalInput")
    eye_d = nc.dram_tensor("eye", [128, 128], F32, kind="ExternalInput")
    ones_d = nc.dram_tensor("ones", [128, 256], FP8, kind="ExternalInput")
    mlo_d = nc.dram_tensor("mlo", [128, 4 * 512], BF16, kind="ExternalInput")
    mhi_d = nc.dram_tensor("mhi", [128, 4 * 512], BF16, kind="ExternalInput")
    stats_d = nc.dram_tensor("stats", [128, 16], F32, kind="ExternalOutput")
    l2m_d = nc.dram_tensor("l2m", [128, 2 * 16], F32, kind="ExternalOutput")
    g2c_d = nc.dram_tensor("g2c", [1, WIN], F32, kind="ExternalOutput")

    with TileContext(nc) as tc:
        with (
            tc.tile_pool(name="consts", bufs=1) as consts,
            tc.tile_pool(name="feat", bufs=1) as featp,
            tc.tile_pool(name="scr", bufs=2) as scrp,
            tc.tile_pool(name="escrp", bufs=4) as escrp,
            tc.tile_pool(name="stat", bufs=2) as statp,
            tc.tile_pool(name="mm", bufs=8, space="PSUM") as mmp,
        ):
            eye = consts.tile([128, 128], F32, tag="eye")
            ones = consts.tile([128, 2, 128], FP8, tag="ones")
            mlo = consts.tile([128, 4, 512], BF16, tag="mlo")
            mhi = consts.tile([128, 4, 512], BF16, tag="mhi")
            nc.gpsimd.dma_start(eye, eye_d[:, :])
            nc.gpsimd.dma_start(ones[:, :, :], ones_d[:, :])

            stats = consts.tile([128, 16], F32, tag="stats")
            l2m = consts.tile([128, 2 * 16], F32, tag="l2m")
            g2crow = consts.tile([1, WIN], F32, tag="g2crow")
            gcbias = consts.tile([128, 1], F32, tag="gcbias")
            nc.vector.memset(gcbias, -GC * INV_TEMP)

            # ---- input tiles ----
            txt = featp.tile([128, NCH, B], BF16, tag="txt")
            img = featp.tile([128, NCH, BPC], BF16, tag="img")
            z = featp.tile([128, NCH, N], FP8, tag="z")

            # Split DMA issue across engines: descriptor generation (~0.5-1us
            # per dma_start) serializes per issuing sequencer. txt/img on SP,
            # z on the Activation engine's queues, consts on gpsimd SWDGE.
            # First chunks in small pieces so phase A starts early.
            # first matmul needs txt c0 piece 0 AND img piece 0 -- interleave
            for p in range(4):
                cs = slice(p * 512, (p + 1) * 512)
                nc.sync.dma_start(txt[:, 0, cs], txt_d[0:128, cs])
                nc.sync.dma_start(img[:, 2 * p:2 * p + 2, :], img_d[:, cs])
            for p in range(4):
                cs = slice(p * 512, (p + 1) * 512)
                nc.sync.dma_start(txt[:, 1, cs], txt_d[128:256, cs])
            for c in range(2, NCH):
                r0 = c * 128
                for p in range(2):
                    cs = slice(p * 1024, (p + 1) * 1024)
                    nc.sync.dma_start(txt[:, c, cs], txt_d[r0:r0 + 128, cs])
            for c in range(NCH):
                r0 = c * 128
                for p in range(2):
                    cs = slice(p * 2048, (p + 1) * 2048)
                    nc.scalar.dma_start(z[:, c, cs], z_d[r0:r0 + 128, cs])
            # masks are needed only by the g2i phase (~60% in) -- load last
            nc.sync.dma_start(mlo[:, :, :], mlo_d[:, :])
            nc.sync.dma_start(mhi[:, :, :], mhi_d[:, :])

            # ================= phase A: l2i side-0 matmuls =================
            # two passes of 4 banks each: pass-0 chains complete as soon as
            # the last txt chunk lands, so the psum->sbuf copies (and the
            # scalar/DVE stats behind them) start ~12us earlier than with all
            # 8 chains open.
            psl = [[None] * NB_L for _ in range(NT_L)]
            cp = []
            for t in range(NT_L):
                cp.append(scrp.tile([128, B], F32, tag="cp", name="cpt",
                                    bufs=2))
            for half in range(2):
                for t in range(NT_L):
                    for b in (2 * half, 2 * half + 1):
                        psl[t][b] = mmp.tile([128, 512], F32, tag="ps",
                                             name="ps")
                for c in range(NCH):
                    for t in range(NT_L):
                        for b in (2 * half, 2 * half + 1):
                            nc.tensor.matmul(
                                psl[t][b],
                                img[:, c, t * 128:(t + 1) * 128],
                                txt[:, c, b * 512:(b + 1) * 512],
                                start=(c == 0), stop=(c == NCH - 1),
                            )
                for t in range(NT_L):
                    for b in (2 * half, 2 * half + 1):
                        nc.vector.tensor_copy(
                            cp[t][:, b * 512:(b + 1) * 512], psl[t][b])

            # ---- l2i side-0 row stats (emitted per g2i iteration, fills gaps)
            # single row-global max + one wide exp instead of 4 per-bank
            # rounds with a combine
            def emit_rowstats(t):
                scr = statp.tile([128, 128], F32, tag="posscr", name="scr")
                nc.gpsimd.tensor_mul(scr, cp[t][:, t * 128:(t + 1) * 128], eye)
                nc.vector.reduce_sum(
                    stats[:, COL_POS_L2I + t:COL_POS_L2I + t + 1],
                    scr, axis=mybir.AxisListType.X)
                gmax = statp.tile([128, 1], F32, tag="gmax", name="gmax")
                nc.vector.reduce_max(gmax, cp[t], axis=mybir.AxisListType.X)
                neggmax = statp.tile([128, 1], F32, tag="neggmax",
                                     name="neggmax")
                nc.vector.tensor_scalar_mul(neggmax, gmax, -ls)
                S = statp.tile([128, 1], F32, tag="S", name="S")
                edump = scrp.tile([128, B], BF16, tag="edump",
                                  name="edump", bufs=2)
                nc.scalar.activation(
                    edump, cp[t], AF.Exp, bias=neggmax, scale=ls,
                    accum_out=S,
                )
                lnS = statp.tile([128, 1], F32, tag="lnS", name="lnS")
                nc.scalar.activation(lnS, S, AF.Ln)
                gms = statp.tile([128, 1], F32, tag="gms", name="gms")
                nc.vector.tensor_scalar_mul(gms, gmax, ls)
                nc.vector.tensor_add(
                    stats[:, COL_LSE_IMG + t:COL_LSE_IMG + t + 1], lnS, gms)

            # ================= g2i (symmetric, window-aligned) =================
            # z arrives pre-normalized (host folds 1/||z|| into the fp8 cast),
            # so psum = sim directly: exp reads PSUM, masks add on PSUM.
            gsum = []
            for t in range(NT_G):
                gsum.append(statp.tile([128, NJ], F32, tag="gsum",
                                       name="gsum", bufs=NT_G))

            # t-outer / q-outer / j-inner: the stationary lhsT (own-row block)
            # stays loaded across 5 consecutive matmuls instead of reloading
            # per matmul. Side-1 transpose groups interleave per t to fill
            # engine gaps.
            escr_tiles = {}  # (j, t//2) -> [128, 2, 512] fp8
            for t in range(NT_G):
                psg = []
                for j in range(NJ):
                    psg.append(mmp.tile([128, 512], F32, tag="ps", name="psg"))
                for q in range(NQ):
                    for j in range(NJ):
                        nc.tensor.matmul(
                            psg[j],
                            z[:, 2 * q:2 * q + 2, t * 128:(t + 1) * 128],
                            z[:, 2 * q:2 * q + 2, j * 512:(j + 1) * 512],
                            start=(q == 0), stop=(q == NQ - 1), perf_mode=DR,
                        )
                for j in range(NJ):
                    if j == NJ - 1:
                        # positive-pair sim: diag at cols t*128 (pre-mask)
                        pscr = statp.tile([128, 128], F32, tag="pscr",
                                          name="pscr")
                        nc.vector.tensor_mul(
                            pscr, psg[j][:, t * 128:(t + 1) * 128], eye)
                        nc.vector.reduce_sum(
                            stats[:, COL_POS_G2I + t:COL_POS_G2I + t + 1],
                            pscr, axis=mybir.AxisListType.X)
                        nc.vector.tensor_add(psg[j], psg[j], mhi[:, t, :])
                    if j == 0:
                        nc.vector.tensor_add(psg[j], psg[j], mlo[:, t, :])
                    key = (j, t // 2)
                    if key not in escr_tiles:
                        escr_tiles[key] = escrp.tile(
                            [128, 2, 512], FP8, tag="escr", name="escr",
                            bufs=10)
                    nc.scalar.activation(
                        escr_tiles[key][:, t % 2, :], psg[j], AF.Exp,
                        scale=INV_TEMP, bias=gcbias,
                        accum_out=gsum[t][:, j:j + 1],
                    )
                # ---- side-1 transposes for this t: 2 col-group tiles ----
                # Within a tile the 4 transposes chain (start only on first):
                # a start=True mid-bank write re-zeroes the whole 2KB region.
                for gp_ in (2 * t, 2 * t + 1):
                    tpg = mmp.tile([128, 512], F32, tag="ps", name="tpg")
                    for half in range(2):
                        g = 2 * gp_ + half
                        for tt in range(NT_L):
                            first = (half == 0 and tt == 0)
                            last = (half == 1 and tt == NT_L - 1)
                            nc.tensor.matmul(
                                tpg[:, half * 256 + tt * 128:
                                    half * 256 + (tt + 1) * 128],
                                cp[tt][:, g * 128:(g + 1) * 128], eye,
                                is_transpose=True, start=first, stop=last,
                                skip_group_check=True,
                            )
                    for half in range(2):
                        g = 2 * gp_ + half
                        sl = slice(half * 256, (half + 1) * 256)
                        nc.vector.reduce_max(
                            l2m[:, g:g + 1], tpg[:, sl],
                            axis=mybir.AxisListType.X)
                        negm = statp.tile([128, 1], F32, tag="negm",
                                          name="negm")
                        nc.vector.tensor_scalar_mul(negm, l2m[:, g:g + 1], -ls)
                        edump2 = scrp.tile([128, 256], BF16, tag="edump2",
                                           name="edump2", bufs=2)
                        nc.scalar.activation(
                            edump2, tpg[:, sl], AF.Exp, bias=negm, scale=ls,
                            accum_out=l2m[:, 16 + g:16 + g + 1])
                if t >= NT_G - NT_L:
                    emit_rowstats(t - (NT_G - NT_L))

            # column sums over all 512 rows: 2 DR ones-matmuls per bank
            for j in range(NJ):
                cps = mmp.tile([128, 512], F32, tag="ps", name="cps")
                for h in range(2):
                    nc.tensor.matmul(
                        cps, ones[:, :, :], escr_tiles[(j, h)][:, :, :],
                        start=(h == 0), stop=(h == 1), perf_mode=DR,
                    )
                nc.vector.tensor_copy(g2crow[:, j * 512:(j + 1) * 512],
                                      cps[0:1, :])

            for t in range(NT_G):
                nc.vector.reduce_sum(
                    stats[:, COL_FWD_G2I + t:COL_FWD_G2I + t + 1],
                    gsum[t], axis=mybir.AxisListType.X)

            nc.sync.dma_start(stats_d[:, :], stats)
            nc.sync.dma_start(l2m_d[:, :], l2m)
            nc.sync.dma_start(g2c_d[:, :], g2crow)

    _split_multi_waits(nc)
    return nc


def _get_program(ls: float) -> bass.Bass:
    key = float(ls)
    if key not in _cache:
        _cache[key] = _build_program(key)
    return _cache[key]


def _make_masks() -> tuple[np.ndarray, np.ndarray]:
    """mlo[t]: -1e30 where window-col w <= 128t+p (backward + self, bank 0).
    mhi[t]: -1e30 where w-2048 >= 128t+p (beyond-window + pos col, bank 4)."""
    bf = ml_dtypes.bfloat16
    p = np.arange(128)[:, None]
    w = np.arange(512)[None, :]
    mlo = np.zeros((128, 4 * 512), dtype=np.float32)
    mhi = np.zeros((128, 4 * 512), dtype=np.float32)
    for t in range(4):
        lr = 128 * t + p
        mlo[:, t * 512:(t + 1) * 512] = np.where(w <= lr, -1e30, 0.0)
        mhi[:, t * 512:(t + 1) * 512] = np.where(w >= lr, -1e30, 0.0)
    return mlo.astype(bf), mhi.astype(bf)


def kernel(image_features, gli_features, text_features, logit_scale):
    ls = float(np.asarray(logit_scale))
    nc = _get_program(ls)

    bf = ml_dtypes.bfloat16
    f8 = ml_dtypes.float8_e4m3fn
    imgT = np.ascontiguousarray(np.asarray(image_features, dtype=np.float32).T)
    txtT = np.ascontiguousarray(np.asarray(text_features, dtype=np.float32).T)
    zT = np.ascontiguousarray(np.concatenate(
        [np.asarray(gli_features, dtype=np.float32),
         np.asarray(image_features, dtype=np.float32)], axis=0).T)
    # fold 1/||z_fp8|| into the fp8 sharding cast: the device GEMM then
    # produces cosine similarities directly
    z8f = zT.astype(f8).astype(np.float32)
    zn8 = (z8f / np.sqrt((z8f * z8f).sum(0, keepdims=True))).astype(f8)

    eye = np.eye(128, dtype=np.float32)
    ones = np.ones((128, 256), dtype=f8)
    mlo, mhi = _make_masks()

    in_maps = []
    for k in range(NCORES):
        txt_k = np.roll(txtT, -BPC * k, axis=1).astype(bf)
        # img swizzled partition-major: [128, NCH*BPC]
        img_k = (imgT[:, BPC * k:BPC * (k + 1)].astype(bf)
                 .reshape(NCH, 128, BPC).transpose(1, 0, 2).reshape(128, -1))
        z_k = np.roll(zn8, -ZPC * k, axis=1)
        in_maps.append({
            "txt": np.ascontiguousarray(txt_k),
            "img": np.ascontiguousarray(img_k),
            "z": np.ascontiguousarray(z_k),
            "eye": eye, "ones": ones, "mlo": mlo, "mhi": mhi,
        })

    res = bass_utils.run_bass_kernel_spmd(nc, in_maps, core_ids=list(range(NCORES)))
    globals()["LAST_RESULT"] = res

    stats = np.stack([r["stats"] for r in res.results]).astype(np.float64)
    l2m = np.stack([r["l2m"] for r in res.results]).astype(np.float64)
    g2c = np.stack([r["g2c"] for r in res.results]).astype(np.float64)[:, 0, :]

    # ---- l2i ----
    lse_img = stats[:, :, COL_LSE_IMG:COL_LSE_IMG + NT_L]  # [8,128,2]
    pos_l2i = stats[:, :, COL_POS_L2I:COL_POS_L2I + NT_L]
    sum_lse_img = lse_img.sum()
    sum_pos = pos_l2i.sum()

    # side-1: combine per-core column partials. core k local col g*128+p is
    # global col (256k + g*128 + p) mod 2048; partial (m, S) covers 256 rows.
    m_part = l2m[:, :, 0:16]    # [8, 128, 16]
    s_part = l2m[:, :, 16:32]
    cols = (np.arange(NCORES)[:, None, None] * BPC
            + np.arange(16)[None, None, :] * 128
            + np.arange(128)[None, :, None]) % B  # [8,128,16]
    m_all = np.zeros((NCORES, B))
    s_all = np.zeros((NCORES, B))
    for k in range(NCORES):
        m_all[k, cols[k].ravel()] = m_part[k].ravel()
        s_all[k, cols[k].ravel()] = s_part[k].ravel()
    m_glob = m_all.max(0)
    lse_txt = ls * m_glob + np.log(
        (s_all * np.exp(ls * (m_all - m_glob[None, :]))).sum(0))
    sum_lse_txt = lse_txt.sum()

    l2i = 0.5 * ((sum_lse_img - ls * sum_pos) / B
                 + (sum_lse_txt - ls * sum_pos) / B)

    # ---- g2i ----
    fwd = stats[:, :, COL_FWD_G2I:COL_FWD_G2I + NT_G]   # [8,128,4] scaled sums
    pos_g2i = stats[:, :, COL_POS_G2I:COL_POS_G2I + NT_G]
    fwd_rows = np.zeros((N,))
    pos_rows = np.zeros((N,))
    rows = (np.arange(NCORES)[:, None, None] * ZPC
            + np.arange(NT_G)[None, None, :] * 128
            + np.arange(128)[None, :, None])  # [8,128,4]
    for k in range(NCORES):
        fwd_rows[rows[k].ravel()] = fwd[k].ravel()
        pos_rows[rows[k].ravel()] = pos_g2i[k].ravel()
    back_rows = np.zeros((N,))
    for k in range(NCORES):
        gcols = (np.arange(WIN) + ZPC * k) % N
        np.add.at(back_rows, gcols, g2c[k])
    # fwd+back are scaled by e^(-GC/T); add pos term and unshift in the log
    total = fwd_rows + back_rows + np.exp((pos_rows - GC) * INV_TEMP)
    lse = np.log(total) + GC * INV_TEMP
    g2i = (lse - pos_rows * INV_TEMP).sum() / N

    tot = l2i + g2i
    return (np.float32(tot), np.float32(l2i), np.float32(g2i))


# revision 39
# speedup vs baseline: 1.1774x; 1.0586x over previous
"""Distributed CLIP-style loss (l2i symmetric CE + g2i NT-Xent) on 8 TRN2 cores.

Strategy (v2): data-parallel row sharding with column-rotated per-core inputs
(rotation = the core's global row offset) so diagonals sit at static local
offsets and one SPMD program serves all 8 cores.

Key optimizations over v1:
 - z ships as fp8-e4m3 (4MB/core instead of 8MB bf16); the g2i GEMM runs in
   fp8 DoubleRow mode (2x PE throughput). Norms are computed on-device from
   fp8 squares via a DoubleRow ones-matmul.
 - g2i exploits symmetry of the similarity matrix: each 128-row tile computes
   only the column window [0, 2560) (its forward half, window-aligned across
   tiles); the backward half comes from column sums of the exp'd forward
   blocks (fp8 ones-matmul), assembled with the row sums on the host.
 - l2i computes the image@text GEMM once. Row-side LSE is computed on-device;
   the text-side (column) LSE partials come from f32 PE-transposes of the
   logits + per-column (max, sum) over the core's 256 rows, combined on host.
 - norm pipeline (fp8 squares -> ones-mm -> recip -> sqrt) is split across
   DVE/gpsimd and interleaved with phase A.
"""

import numpy as np
import ml_dtypes

import concourse.bass as bass
import concourse.mybir as mybir
from concourse.tile import TileContext
from concourse import bass_utils


# --- compat patches for the walrus build in this container ---------------
def _sem_clear_compat(self, sem):
    nums = list(sem) if isinstance(sem, range) else [
        sem.num if hasattr(sem, "num") else int(sem)
    ]
    last = None
    for n in nums:
        last = self.add_instruction(
            mybir.InstEventSemaphore(
                name=self.bass.get_next_instruction_name(),
                ins=[], outs=[],
                sync_info=mybir.SyncInfo(
                    on_wait=[],
                    on_update=[mybir.SyncUpdate(
                        sync_type="semaphore", id=n,
                        update_mode="sem-wr-imm", update_value=0)],
                ),
            )
        )
    return last


bass.BassGpSimd.sem_clear = _sem_clear_compat

_mw_ctr = [0]


def _split_multi_waits(nc: bass.Bass) -> None:
    for f in nc.m.functions:
        for bb in f.blocks:
            out = []
            changed = False
            for inst in bb.instructions:
                si = inst.sync_info
                waits = list(si.on_wait) if si is not None and si.on_wait else []
                if len(waits) > 1:
                    for w in waits[:-1]:
                        _mw_ctr[0] += 1
                        es = mybir.InstEventSemaphore(
                            name=f"I-mwsplit-{_mw_ctr[0]}",
                            engine=inst.engine,
                            ins=[], outs=[],
                            sync_info=mybir.SyncInfo(on_wait=[w], on_update=[]),
                        )
                        out.append(es)
                    inst.sync_info = mybir.SyncInfo(
                        on_wait=[waits[-1]],
                        on_update=list(si.on_update or []),
                    )
                    changed = True
                out.append(inst)
            if changed:
                bb.instructions = out
# -------------------------------------------------------------------------

B = 2048
D = 1024
N = 2 * B                 # 4096 z rows
NCORES = 8
TEMP = 0.05
INV_TEMP = 1.0 / TEMP
GC = 0.25                 # g2i exp shift: e^((sim-GC)/T) keeps fp8 escr in range
BPC = B // NCORES         # 256 image/text rows per core
ZPC = N // NCORES         # 512 z rows per core
NCH = D // 128            # 8 contraction chunks
NQ = NCH // 2             # 4 DoubleRow chunk-pairs
WIN = 2560                # g2i forward window per row-tile (5 banks of 512)
NJ = WIN // 512           # 5 g2i column banks
NT_G = ZPC // 128         # 4 g2i row-tiles per core
NT_L = BPC // 128         # 2 l2i row-tiles per core
NB_L = B // 512           # 4 l2i column banks

BF16 = mybir.dt.bfloat16
F32 = mybir.dt.float32
FP8 = mybir.dt.float8e4
AF = mybir.ActivationFunctionType
DR = mybir.MatmulPerfMode.DoubleRow

# stats column layout ([128, 16] f32 per core)
COL_LSE_IMG = 0   # +t (2): complete row lse (ls*gmax + ln S)
COL_POS_L2I = 2   # +t (2): raw positive dot (unscaled)
COL_FWD_G2I = 4   # +t (4): forward exp-sums, scaled by e^(-GC/T)
COL_POS_G2I = 8   # +t (4): positive-pair cosine sim

_cache: dict = {}


def _build_program(ls: float) -> bass.Bass:
    nc = bass.Bass(trn_type="TRN2")
    txt_d = nc.dram_tensor("txt", [D, B], BF16, kind="ExternalInput")
    # img is host-swizzled to partition-major [128, NCH*BPC] -> one DMA
    img_d = nc.dram_tensor("img", [128, NCH * BPC], BF16, kind="ExternalInput")
    z_d = nc.dram_tensor("z", [D, N], FP8, kind="ExternalInput")
    eye_d = nc.dram_tensor("eye", [128, 128], F32, kind="ExternalInput")
    ones_d = nc.dram_tensor("ones", [128, 256], FP8, kind="ExternalInput")
    mlo_d = nc.dram_tensor("mlo", [128, 4 * 512], BF16, kind="ExternalInput")
    mhi_d = nc.dram_tensor("mhi", [128, 4 * 512], BF16, kind="ExternalInput")
    stats_d = nc.dram_tensor("stats", [128, 16], F32, kind="ExternalOutput")
    l2m_d = nc.dram_tensor("l2m", [128, 2 * 16], F32, kind="ExternalOutput")
    g2c_d = nc.dram_tensor("g2c", [1, WIN], F32, kind="ExternalOutput")

    with TileContext(nc) as tc:
        with (
            tc.tile_pool(name="consts", bufs=1) as consts,
            tc.tile_pool(name="feat", bufs=1) as featp,
            tc.tile_pool(name="scr", bufs=2) as scrp,
            tc.tile_pool(name="escrp", bufs=4) as escrp,
            tc.tile_pool(name="stat", bufs=2) as statp,
            tc.tile_pool(name="mm", bufs=8, space="PSUM") as mmp,
        ):
            eye = consts.tile([128, 128], F32, tag="eye")
            ones = consts.tile([128, 2, 128], FP8, tag="ones")
            mlo = consts.tile([128, 4, 512], BF16, tag="mlo")
            mhi = consts.tile([128, 4, 512], BF16, tag="mhi")
            nc.gpsimd.dma_start(eye, eye_d[:, :])
            nc.gpsimd.dma_start(ones[:, :, :], ones_d[:, :])

            stats = consts.tile([128, 16], F32, tag="stats")
            l2m = consts.tile([128, 2 * 16], F32, tag="l2m")
            g2crow = consts.tile([1, WIN], F32, tag="g2crow")
            gcbias = consts.tile([128, 1], F32, tag="gcbias")
            nc.vector.memset(gcbias, -GC * INV_TEMP)

            # ---- input tiles ----
            txt = featp.tile([128, NCH, B], BF16, tag="txt")
            img = featp.tile([128, NCH, BPC], BF16, tag="img")
            z = featp.tile([128, NCH, N], FP8, tag="z")

            # Split DMA issue across engines: descriptor generation (~0.5-1us
            # per dma_start) serializes per issuing sequencer. txt/img on SP,
            # z on the Activation engine's queues, consts on gpsimd SWDGE.
            # First chunks in small pieces so phase A starts early.
            # first matmul needs txt c0 piece 0 AND img piece 0 -- interleave
            for p in range(4):
                cs = slice(p * 512, (p + 1) * 512)
                nc.sync.dma_start(txt[:, 0, cs], txt_d[0:128, cs])
                nc.sync.dma_start(img[:, 2 * p:2 * p + 2, :], img_d[:, cs])
            for p in range(4):
                cs = slice(p * 512, (p + 1) * 512)
                nc.sync.dma_start(txt[:, 1, cs], txt_d[128:256, cs])
            for c in range(2, NCH):
                r0 = c * 128
                for p in range(2):
                    cs = slice(p * 1024, (p + 1) * 1024)
                    nc.sync.dma_start(txt[:, c, cs], txt_d[r0:r0 + 128, cs])
            for c in range(NCH):
                r0 = c * 128
                for p in range(2):
                    cs = slice(p * 2048, (p + 1) * 2048)
                    nc.scalar.dma_start(z[:, c, cs], z_d[r0:r0 + 128, cs])
            # masks are needed only by the g2i phase (~60% in) -- load last
            nc.sync.dma_start(mlo[:, :, :], mlo_d[:, :])
            nc.sync.dma_start(mhi[:, :, :], mhi_d[:, :])

            # ================= phase A: l2i side-0 matmuls =================
            # two passes of 4 banks each: pass-0 chains complete as soon as
            # the last txt chunk lands, so the psum->sbuf copies (and the
            # scalar/DVE stats behind them) start ~12us earlier than with all
            # 8 chains open.
            psl = [[None] * NB_L for _ in range(NT_L)]
            cp = []
            for t in range(NT_L):
                cp.append(scrp.tile([128, B], F32, tag="cp", name="cpt",
                                    bufs=2))
            for half in range(2):
                for t in range(NT_L):
                    for b in (2 * half, 2 * half + 1):
                        psl[t][b] = mmp.tile([128, 512], F32, tag="ps",
                                             name="ps")
                for c in range(NCH):
                    for t in range(NT_L):
                        for b in (2 * half, 2 * half + 1):
                            nc.tensor.matmul(
                                psl[t][b],
                                img[:, c, t * 128:(t + 1) * 128],
                                txt[:, c, b * 512:(b + 1) * 512],
                                start=(c == 0), stop=(c == NCH - 1),
                            )
                for t in range(NT_L):
                    for b in (2 * half, 2 * half + 1):
                        nc.vector.tensor_copy(
                            cp[t][:, b * 512:(b + 1) * 512], psl[t][b])

            # ---- l2i side-0 row stats (emitted per g2i iteration, fills gaps)
            # single row-global max + one wide exp instead of 4 per-bank
            # rounds with a combine
            def emit_rowstats(t):
                scr = statp.tile([128, 128], F32, tag="posscr", name="scr")
                nc.gpsimd.tensor_mul(scr, cp[t][:, t * 128:(t + 1) * 128], eye)
                nc.vector.reduce_sum(
                    stats[:, COL_POS_L2I + t:COL_POS_L2I + t + 1],
                    scr, axis=mybir.AxisListType.X)
                gmax = statp.tile([128, 1], F32, tag="gmax", name="gmax")
                nc.vector.reduce_max(gmax, cp[t], axis=mybir.AxisListType.X)
                neggmax = statp.tile([128, 1], F32, tag="neggmax",
                                     name="neggmax")
                nc.vector.tensor_scalar_mul(neggmax, gmax, -ls)
                S = statp.tile([128, 1], F32, tag="S", name="S")
                edump = scrp.tile([128, B], BF16, tag="edump",
                                  name="edump", bufs=2)
                nc.scalar.activation(
                    edump, cp[t], AF.Exp, bias=neggmax, scale=ls,
                    accum_out=S,
                )
                lnS = statp.tile([128, 1], F32, tag="lnS", name="lnS")
                nc.scalar.activation(lnS, S, AF.Ln)
                gms = statp.tile([128, 1], F32, tag="gms", name="gms")
                nc.vector.tensor_scalar_mul(gms, gmax, ls)
                nc.vector.tensor_add(
                    stats[:, COL_LSE_IMG + t:COL_LSE_IMG + t + 1], lnS, gms)

            # ================= g2i (symmetric, window-aligned) =================
            # z arrives pre-normalized (host folds 1/||z|| into the fp8 cast),
            # so psum = sim directly: exp reads PSUM, masks add on PSUM.
            gsum = []
            for t in range(NT_G):
                gsum.append(statp.tile([128, NJ], F32, tag="gsum",
                                       name="gsum", bufs=NT_G))

            # t-outer / q-outer / j-inner: the stationary lhsT (own-row block)
            # stays loaded across 5 consecutive matmuls instead of reloading
            # per matmul. Side-1 transpose groups interleave per t to fill
            # engine gaps.
            escr_tiles = {}  # (j, t//2) -> [128, 2, 512] fp8
            for t in range(NT_G):
                psg = []
                for j in range(NJ):
                    psg.append(mmp.tile([128, 512], F32, tag="ps", name="psg"))
                for q in range(NQ):
                    for j in range(NJ):
                        nc.tensor.matmul(
                            psg[j],
                            z[:, 2 * q:2 * q + 2, t * 128:(t + 1) * 128],
                            z[:, 2 * q:2 * q + 2, j * 512:(j + 1) * 512],
                            start=(q == 0), stop=(q == NQ - 1), perf_mode=DR,
                        )
                for j in range(NJ):
                    if j == NJ - 1:
                        # positive-pair sim: diag at cols t*128 (pre-mask)
                        pscr = statp.tile([128, 128], F32, tag="pscr",
                                          name="pscr")
                        nc.vector.tensor_mul(
                            pscr, psg[j][:, t * 128:(t + 1) * 128], eye)
                        nc.vector.reduce_sum(
                            stats[:, COL_POS_G2I + t:COL_POS_G2I + t + 1],
                            pscr, axis=mybir.AxisListType.X)
                        nc.vector.tensor_add(psg[j], psg[j], mhi[:, t, :])
                    if j == 0:
                        nc.vector.tensor_add(psg[j], psg[j], mlo[:, t, :])
                    key = (j, t // 2)
                    if key not in escr_tiles:
                        escr_tiles[key] = escrp.tile(
                            [128, 2, 512], FP8, tag="escr", name="escr",
                            bufs=10)
                    nc.scalar.activation(
                        escr_tiles[key][:, t % 2, :], psg[j], AF.Exp,
                        scale=INV_TEMP, bias=gcbias,
                        accum_out=gsum[t][:, j:j + 1],
                    )
                # ---- side-1 transposes for this t: 2 col-group tiles ----
                # Within a tile the 4 transposes chain (start only on first):
                # a start=True mid-bank write re-zeroes the whole 2KB region.
                for gp_ in (2 * t, 2 * t + 1):
                    tpg = mmp.tile([128, 512], F32, tag="ps", name="tpg")
                    for half in range(2):
                        g = 2 * gp_ + half
                        for tt in range(NT_L):
                            first = (half == 0 and tt == 0)
                            last = (half == 1 and tt == NT_L - 1)
                            nc.tensor.matmul(
                                tpg[:, half * 256 + tt * 128:
                                    half * 256 + (tt + 1) * 128],
                                cp[tt][:, g * 128:(g + 1) * 128], eye,
                                is_transpose=True, start=first, stop=last,
                                skip_group_check=True,
                            )
                    # per-column max of both groups, then one wide exp with a
                    # per-group bias trick: bias must be per-partition, and
                    # the two groups have different maxes -> keep per-group
                    # exps for max subtraction but sum on DVE (cheaper than
                    # the scalar accumulator readout)
                    g0 = 2 * gp_
                    nc.vector.reduce_max(
                        l2m[:, g0:g0 + 1], tpg[:, 0:256],
                        axis=mybir.AxisListType.X)
                    nc.vector.reduce_max(
                        l2m[:, g0 + 1:g0 + 2], tpg[:, 256:512],
                        axis=mybir.AxisListType.X)
                    edump2 = scrp.tile([128, 2, 256], BF16, tag="edump2",
                                       name="edump2", bufs=2)
                    for half in range(2):
                        g = 2 * gp_ + half
                        negm = statp.tile([128, 1], F32, tag="negm",
                                          name="negm")
                        nc.vector.tensor_scalar_mul(negm, l2m[:, g:g + 1], -ls)
                        nc.scalar.activation(
                            edump2[:, half, :],
                            tpg[:, half * 256:(half + 1) * 256],
                            AF.Exp, bias=negm, scale=ls)
                    nc.vector.tensor_reduce(
                        l2m[:, 16 + g0:16 + g0 + 2], edump2[:, :, :],
                        op=mybir.AluOpType.add, axis=mybir.AxisListType.X)
                if t >= NT_G - NT_L:
                    emit_rowstats(t - (NT_G - NT_L))

            # column sums over all 512 rows: 2 DR ones-matmuls per bank
            for j in range(NJ):
                cps = mmp.tile([128, 512], F32, tag="ps", name="cps")
                for h in range(2):
                    nc.tensor.matmul(
                        cps, ones[:, :, :], escr_tiles[(j, h)][:, :, :],
                        start=(h == 0), stop=(h == 1), perf_mode=DR,
                    )
                nc.vector.tensor_copy(g2crow[:, j * 512:(j + 1) * 512],
                                      cps[0:1, :])

            for t in range(NT_G):
                nc.vector.reduce_sum(
                    stats[:, COL_FWD_G2I + t:COL_FWD_G2I + t + 1],
                    gsum[t], axis=mybir.AxisListType.X)

            nc.sync.dma_start(stats_d[:, :], stats)
            nc.sync.dma_start(l2m_d[:, :], l2m)
            nc.sync.dma_start(g2c_d[:, :], g2crow)

    _split_multi_waits(nc)
    return nc


def _get_program(ls: float) -> bass.Bass:
    key = float(ls)
    if key not in _cache:
        _cache[key] = _build_program(key)
    return _cache[key]


def _make_masks() -> tuple[np.ndarray, np.ndarray]:
    """mlo[t]: -1e30 where window-col w <= 128t+p (backward + self, bank 0).
    mhi[t]: -1e30 where w-2048 >= 128t+p (beyond-window + pos col, bank 4)."""
    bf = ml_dtypes.bfloat16
    p = np.arange(128)[:, None]
    w = np.arange(512)[None, :]
    mlo = np.zeros((128, 4 * 512), dtype=np.float32)
    mhi = np.zeros((128, 4 * 512), dtype=np.float32)
    for t in range(4):
        lr = 128 * t + p
        mlo[:, t * 512:(t + 1) * 512] = np.where(w <= lr, -1e30, 0.0)
        mhi[:, t * 512:(t + 1) * 512] = np.where(w >= lr, -1e30, 0.0)
    return mlo.astype(bf), mhi.astype(bf)


def kernel(image_features, gli_features, text_features, logit_scale):
    ls = float(np.asarray(logit_scale))
    nc = _get_program(ls)

    bf = ml_dtypes.bfloat16
    f8 = ml_dtypes.float8_e4m3fn
    imgT = np.ascontiguousarray(np.asarray(image_features, dtype=np.float32).T)
    txtT = np.ascontiguousarray(np.asarray(text_features, dtype=np.float32).T)
    zT = np.ascontiguousarray(np.concatenate(
        [np.asarray(gli_features, dtype=np.float32),
         np.asarray(image_features, dtype=np.float32)], axis=0).T)
    # fold 1/||z_fp8|| into the fp8 sharding cast: the device GEMM then
    # produces cosine similarities directly
    z8f = zT.astype(f8).astype(np.float32)
    zn8 = (z8f / np.sqrt((z8f * z8f).sum(0, keepdims=True))).astype(f8)

    eye = np.eye(128, dtype=np.float32)
    ones = np.ones((128, 256), dtype=f8)
    mlo, mhi = _make_masks()

    in_maps = []
    for k in range(NCORES):
        txt_k = np.roll(txtT, -BPC * k, axis=1).astype(bf)
        # img swizzled partition-major: [128, NCH*BPC]
        img_k = (imgT[:, BPC * k:BPC * (k + 1)].astype(bf)
                 .reshape(NCH, 128, BPC).transpose(1, 0, 2).reshape(128, -1))
        z_k = np.roll(zn8, -ZPC * k, axis=1)
        in_maps.append({
            "txt": np.ascontiguousarray(txt_k),
            "img": np.ascontiguousarray(img_k),
            "z": np.ascontiguousarray(z_k),
            "eye": eye, "ones": ones, "mlo": mlo, "mhi": mhi,
        })

    res = bass_utils.run_bass_kernel_spmd(nc, in_maps, core_ids=list(range(NCORES)))
    globals()["LAST_RESULT"] = res

    stats = np.stack([r["stats"] for r in res.results]).astype(np.float64)
    l2m = np.stack([r["l2m"] for r in res.results]).astype(np.float64)
    g2c = np.stack([r["g2c"] for r in res.results]).astype(np.float64)[:, 0, :]

    # ---- l2i ----
    lse_img = stats[:, :, COL_LSE_IMG:COL_LSE_IMG + NT_L]  # [8,128,2]
    pos_l2i = stats[:, :, COL_POS_L2I:COL_POS_L2I + NT_L]
    sum_lse_img = lse_img.sum()
    sum_pos = pos_l2i.sum()

    # side-1: combine per-core column partials. core k local col g*128+p is
    # global col (256k + g*128 + p) mod 2048; partial (m, S) covers 256 rows.
    m_part = l2m[:, :, 0:16]    # [8, 128, 16]
    s_part = l2m[:, :, 16:32]
    cols = (np.arange(NCORES)[:, None, None] * BPC
            + np.arange(16)[None, None, :] * 128
            + np.arange(128)[None, :, None]) % B  # [8,128,16]
    m_all = np.zeros((NCORES, B))
    s_all = np.zeros((NCORES, B))
    for k in range(NCORES):
        m_all[k, cols[k].ravel()] = m_part[k].ravel()
        s_all[k, cols[k].ravel()] = s_part[k].ravel()
    m_glob = m_all.max(0)
    lse_txt = ls * m_glob + np.log(
        (s_all * np.exp(ls * (m_all - m_glob[None, :]))).sum(0))
    sum_lse_txt = lse_txt.sum()

    l2i = 0.5 * ((sum_lse_img - ls * sum_pos) / B
                 + (sum_lse_txt - ls * sum_pos) / B)

    # ---- g2i ----
    fwd = stats[:, :, COL_FWD_G2I:COL_FWD_G2I + NT_G]   # [8,128,4] scaled sums
    pos_g2i = stats[:, :, COL_POS_G2I:COL_POS_G2I + NT_G]
    fwd_rows = np.zeros((N,))
    pos_rows = np.zeros((N,))
    rows = (np.arange(NCORES)[:, None, None] * ZPC
            + np.arange(NT_G)[None, None, :] * 128
            + np.arange(128)[None, :, None])  # [8,128,4]
    for k in range(NCORES):
        fwd_rows[rows[k].ravel()] = fwd[k].ravel()
        pos_rows[rows[k].ravel()] = pos_g2i[k].ravel()
    back_rows = np.zeros((N,))
    for k in range(NCORES):
        gcols = (np.arange(WIN) + ZPC * k) % N
        np.add.at(back_rows, gcols, g2c[k])
    # fwd+back are scaled by e^(-GC/T); add pos term and unshift in the log
    total = fwd_rows + back_rows + np.exp((pos_rows - GC) * INV_TEMP)
    lse = np.log(total) + GC * INV_TEMP
    g2i = (lse - pos_rows * INV_TEMP).sum() / N

    tot = l2i + g2i
    return (np.float32(tot), np.float32(l2i), np.float32(g2i))
